# revision 10
# baseline (speedup 1.0000x reference)
"""Trainium2 Bass kernel for CNF log-prob (nn_CNF_86019605004441).

Reference computation (per batch row b of B=32768):
  Integrate (z, logp) from t=1 to t=0 with 4 fixed RK4 steps (steps=5 ->
  4 intervals). Each RK4 stage evaluates
     f(t, z)   = tanh([z, ctx, t] @ W1 + b1) @ W2 + b2
     div(t, z) = eps^T J eps  (Hutchinson, exact via jvp)
  With h = tanh(a):  div = sum_j (1 - h_j^2) * t1_j * v_j
     where t1 = eps @ W1[:16]  and  v = eps @ W2^T  are eval-independent.
  Using u = t1*v and U = sum_j u_j:  div = U - S,  S = sum_j h_j^2 u_j.
  logp(x) = -0.5*sum(z1^2) - 0.5*16*log(2pi) + delta_logp.

Sharding: pure data parallel, batch 32768 -> 8 cores x 4096 rows.

On-core layout (features on partitions, batch on the free axis), v2:
  Two mm1 input buffers inTa/inTb [98, 4096] f32r: rows 0-15 z, 16-32
  scratch (row 32 = logp in inTa), 33-95 ctx, 96 ones, 97 last ctx row.
  inTa holds the step-start state (z_n, logp_n); stages 1-3 read/write
  inTb as the stage input (z + alpha*k).

  Per eval (16 total), per unit (8 units of 512 batch cols), pipelined
  with an emission lag so the in-order PE queue never head-blocks:
    mm1: 4 chunk matmuls [98,128]^T @ inT -> pa psum (2 half tiles)
    tanh -> h fp16 (ACT, 2 ops);  hh = h*h (DVE);  q = hh*u (DVE)
    mm2: f = W2^T h -> CUR[0:32], S = ones^T q -> CUR[32] (8 fp16
         matmuls into one 1-bank psum tile; no ACC matmuls)
    RK4 state updates as scalar_tensor_tensor on GPSIMD (Pool):
      stages 0-2: inTb = alpha*CUR + inTa ; acc = w*CUR + (inTa|acc)
      stage 3:    inTa = w3*CUR + acc     (z_{n+1}, logp_{n+1})
  (logp row: CUR[32] = +S; div = U - S telescopes the U term into the
   logp init, exactly as v1.)
Finalize: zsq = (z1 - b2)^2 ; colsum via ones-matmul ; out = -0.5*colsum
  + inTa[32].
"""

import sys
from collections import deque
import numpy as np

for _p in ("/opt/trn_rl_repo",):
    if _p not in sys.path:
        sys.path.insert(0, _p)

DIM, COND, HID = 16, 64, 512
B, NCORES = 32768, 8
NB = B // NCORES          # 4096 batch rows per core
P = 128                   # partitions
NCH = HID // P            # 4 hidden chunks
NJ = NB // 512            # 8 batch column groups (units)
NSCR = 17                 # scratch rows 16..32 (div lands at 32)
KIN = DIM + NSCR + COND + 1  # 98 stationary rows
FD_P = DIM + NSCR            # 33 = fd/state partition rows
CTX0 = DIM + NSCR            # ctx rows 33..95 + row 97 (96 is the ones row)
ONE_R = 96                   # ones row
DV = DIM + NSCR - 1          # 32 = divergence / logp row
NSTEPS, NSTAGE = 4, 4
NEV = NSTEPS * NSTAGE     # 16 rhs evaluations
LOG2PI = float(np.log(2.0 * np.pi))
LAG = 4                   # software-pipeline emission lag (units)
# hh engine per unit: pool 5/8, act 2/8, dve 1/8 (TensorScalarPtr is not
# a legal Pool opcode, so Pool pays the 0.42 Multiply efficiency)
HHE = ["pool", "act", "pool", "dve", "pool", "act", "pool", "pool"]


def _schedule():
    """Per-eval (t, alpha_next, w, delta) for classic RK4, t:1->0, dt=-0.25."""
    ts = np.linspace(1.0, 0.0, NSTEPS + 1)
    evs = []
    for s in range(NSTEPS):
        t0 = float(ts[s])
        dt = float(ts[s + 1] - ts[s])
        dbase = s * dt
        evs.append(dict(t=t0, alpha=dt / 2, w=dt / 6, delta=dbase))
        evs.append(dict(t=t0 + dt / 2, alpha=dt / 2, w=dt / 3, delta=dbase + dt / 2))
        evs.append(dict(t=t0 + dt / 2, alpha=dt, w=dt / 3, delta=dbase + dt / 2))
        evs.append(dict(t=t0 + dt, alpha=None, w=dt / 6, delta=dbase + dt))
    return evs


def prep_host_inputs(x, context, eps, W1, b1, W2, b2):
    """Host-side layout prep (transposes + per-eval stationary weight packing).

    Returns the in_map dict for one core given that core's batch slice."""
    evs = _schedule()
    W1 = np.asarray(W1, np.float32)
    b1 = np.asarray(b1, np.float32)
    W2 = np.asarray(W2, np.float32)
    b2 = np.asarray(b2, np.float32)

    gz = W1[:DIM].T @ b2  # [512], the z-column correction for deferred b2
    W1v = np.zeros((KIN, NEV * NCH, P), np.float32)
    for i, ev in enumerate(evs):
        for c in range(NCH):
            sl = slice(c * P, (c + 1) * P)
            v = i * NCH + c
            W1v[0:DIM, v, :] = W1[0:DIM, sl]
            # rows DIM..DIM+NSCR-1 stay zero: scratch rows of inT
            W1v[CTX0:ONE_R, v, :] = W1[DIM : DIM + COND - 1, sl]
            W1v[KIN - 1, v, :] = W1[DIM + COND - 1, sl]
            W1v[ONE_R, v, :] = (
                ev["t"] * W1[DIM + COND, sl] + b1[sl] + ev["delta"] * gz[sl]
            )

    W2f16 = np.zeros((P, NCH, 32), np.float16)
    W2f16[:, :, :DIM] = W2.reshape(NCH, P, DIM).transpose(1, 0, 2).astype(np.float16)
    onesW = np.ones((P, 1), np.float16)
    W2T = np.ascontiguousarray(W2.T)  # [16, 512] for the v = eps@W2^T matmul
    b2c = (4 * (-0.25)) * b2.reshape(DIM, 1).astype(np.float32)  # D_final*b2

    def core_map(xs, cs, es):
        initT = np.zeros((KIN, NB), np.float32)
        initT[0:DIM] = xs.T
        initT[CTX0:ONE_R] = cs.T[0 : COND - 1]
        initT[KIN - 1] = cs.T[COND - 1]
        initT[ONE_R] = 1.0
        return {
            "initT": initT,                          # [98, NB]
            "epsT": np.ascontiguousarray(es.T),     # [16, NB]
            "onesZ": np.ones((DIM, 1), np.float32),
            "W1v": W1v,                              # [98, 64, 128]
            "W2T": W2T,                              # [16, 512]
            "W2f16": W2f16,                          # [128, 4, 32]
            "onesW": onesW,                          # [128, 1]
            "b2c": b2c,                              # [16, 1]
        }

    return [
        core_map(
            np.asarray(x, np.float32)[i * NB : (i + 1) * NB],
            np.asarray(context, np.float32)[i * NB : (i + 1) * NB],
            np.asarray(eps, np.float32)[i * NB : (i + 1) * NB],
        )
        for i in range(NCORES)
    ]


def build(nc, tc, ctx):
    """Emit the kernel into TileContext tc (single SPMD program, all cores)."""
    import concourse.bass as bass
    from concourse import mybir

    f32 = mybir.dt.float32
    f32r = mybir.dt.float32r
    f16 = mybir.dt.float16
    AF = mybir.ActivationFunctionType
    OP = mybir.AluOpType
    evs = _schedule()

    initT = nc.dram_tensor("initT", [KIN, NB], f32r, kind="ExternalInput").ap()
    epsT = nc.dram_tensor("epsT", [DIM, NB], f32r, kind="ExternalInput").ap()
    onesZ_d = nc.dram_tensor("onesZ", [DIM, 1], f32r, kind="ExternalInput").ap()
    W1v_d = nc.dram_tensor("W1v", [KIN, NEV * NCH, P], f32r, kind="ExternalInput").ap()
    W2T_d = nc.dram_tensor("W2T", [DIM, HID], f32r, kind="ExternalInput").ap()
    W2f_d = nc.dram_tensor("W2f16", [P, NCH, 32], f16, kind="ExternalInput").ap()
    onesW_d = nc.dram_tensor("onesW", [P, 1], f16, kind="ExternalInput").ap()
    b2c_d = nc.dram_tensor("b2c", [DIM, 1], f32, kind="ExternalInput").ap()
    out_d = nc.dram_tensor("out", [1, NB], f32, kind="ExternalOutput").ap()

    const = ctx.enter_context(tc.tile_pool(name="const", bufs=1))
    state = ctx.enter_context(tc.tile_pool(name="state", bufs=1))
    work = ctx.enter_context(tc.tile_pool(name="work", bufs=4))
    pa_pool = ctx.enter_context(tc.tile_pool(name="pa", bufs=2, space="PSUM"))
    fd_pool = ctx.enter_context(tc.tile_pool(name="fd", bufs=4, space="PSUM"))

    # ---- persistent SBUF ----
    inTa = state.tile([KIN, NB], f32r)   # step-start state + ctx/ones
    inTb = state.tile([KIN, NB], f32r)   # stage input (z + alpha*k) + ctx/ones
    acc = state.tile([FD_P, NB], f32)    # RK4 weighted accumulator
    u = state.tile([P, NCH, NB], f16)
    W1v = const.tile([KIN, NEV * NCH, P], f32r)
    W2T = const.tile([DIM, HID], f32r)
    W2f = const.tile([P, NCH, 32], f16)
    onesW = const.tile([P, 1], f16)
    ones16 = const.tile([P, 1], f16)
    onesZ = const.tile([DIM, 1], f32r)
    b2c = const.tile([DIM, 1], f32)
    ept = const.tile([DIM, NB], f32r)

    nc.gpsimd.dma_start(inTa[:, :], initT)
    nc.gpsimd.dma_start(inTb[:, :], initT)
    nc.gpsimd.dma_start(onesZ[:], onesZ_d)
    nc.gpsimd.dma_start(ept[:], epsT)
    nc.gpsimd.dma_start(W1v[:], W1v_d)
    nc.gpsimd.dma_start(W2T[:], W2T_d)
    nc.gpsimd.dma_start(W2f[:], W2f_d)
    nc.gpsimd.dma_start(onesW[:], onesW_d)
    nc.gpsimd.dma_start(b2c[:], b2c_d)
    nc.vector.memset(ones16[:], 1.0)

    # ---- precompute u = (eps@W1z) * (eps@W2^T), transposed layout ----
    for qt in range(4):
        for c in range(NCH):
            js = slice(qt * (NB // 4), (qt + 1) * (NB // 4))
            pt1 = pa_pool.tile([P, 2, 512], f32, tag="pa")
            pt2 = pa_pool.tile([P, 2, 512], f32, tag="pa")
            for n in range(2):
                cs = slice((qt * 2 + n) * 512, (qt * 2 + n + 1) * 512)
                nc.tensor.matmul(
                    pt1[:, n, :], W1v[0:DIM, c, :], ept[:, cs], start=True, stop=True
                )
                nc.tensor.matmul(
                    pt2[:, n, :], W2T[:, c * P : (c + 1) * P], ept[:, cs],
                    start=True, stop=True,
                )
            usl = u[:, c, js].rearrange("p (a b) -> p a b", a=2)
            nc.scalar.activation(usl, pt1[:, :, :], AF.Copy)
            nc.vector.tensor_tensor(usl, usl, pt2[:, :, :], op=OP.mult)

    # ---- U = colsum(u) -> inTa row 32 = U - 0.5*DIM*log(2pi) ----
    for j in range(NJ):
        js = slice(j * 512, (j + 1) * 512)
        pU = fd_pool.tile([1, 512], f32, tag="fd")
        for c in range(NCH):
            nc.tensor.matmul(
                pU[:, :], ones16[:], u[:, c, js], start=(c == 0), stop=(c == NCH - 1)
            )
        nc.scalar.activation(
            inTa[DV : DV + 1, js], pU[:, :], AF.Copy, bias=-0.5 * DIM * LOG2PI
        )

    # ---- main loop: 16 evals x 8 units, software-pipelined emission ----
    hq = {}

    def emit_front(evi, uu):
        """mm1 -> tanh -> hh -> q for (eval evi, unit uu)."""
        js = slice(uu * 512, (uu + 1) * 512)
        src = inTa if evi % NSTAGE == 0 else inTb
        h = work.tile([P, NCH, 512], f16, tag="h", bufs=5)
        for half in range(2):
            pa = pa_pool.tile([P, 2, 512], f32, tag="pa")
            for n in range(2):
                c = half * 2 + n
                nc.tensor.matmul(
                    pa[:, n, :], W1v[:, evi * NCH + c, :], src[:, js],
                    start=True, stop=True,
                )
            nc.scalar.activation(
                h[:, half * 2 : half * 2 + 2, :], pa[:, :, :], AF.Tanh
            )
        # hh engine is balanced statically: Pool (SBUF-only op, so GPSIMD is
        # legal) takes half the units, ACT-Square ~2.5/8, DVE the rest; q
        # (tensor*tensor) runs on DVE in the 2x fp16 mode.
        hht = work.tile([P, NCH, 512], f16, tag="hh", bufs=3)
        he = HHE[uu] if HHE[uu] != "x" else ("act" if evi % 2 == 0 else "dve")
        if he == "act":
            nc.scalar.activation(hht[:, :, :], h[:, :, :], AF.Square)
        elif he == "pool":
            nc.gpsimd.tensor_tensor(hht[:, :, :], h[:, :, :], h[:, :, :], op=OP.mult)
        else:
            nc.vector.tensor_tensor(hht[:, :, :], h[:, :, :], h[:, :, :], op=OP.mult)
        q = work.tile([P, NCH, 512], f16, tag="q", bufs=5)
        nc.vector.tensor_tensor(q[:, :, :], hht[:, :, :], u[:, :, js], op=OP.mult)
        hq[(evi, uu)] = (h, q)

    def emit_back(evi, uu):
        """mm2 (f + S) -> RK4 state STTs for (eval evi, unit uu)."""
        js = slice(uu * 512, (uu + 1) * 512)
        h, q = hq.pop((evi, uu))
        cur = fd_pool.tile([FD_P, 512], f32, tag="fd")
        for c in range(NCH):
            st, sp = c == 0, c == NCH - 1
            nc.tensor.matmul(
                cur[0:32, :], W2f[:, c, :], h[:, c, :],
                start=st, stop=sp, skip_group_check=True,
            )
            nc.tensor.matmul(
                cur[DV : DV + 1, :], onesW[:, 0:1], q[:, c, :],
                start=st, stop=sp, skip_group_check=True,
            )
        ev = evs[evi]
        stage = evi % NSTAGE
        if stage < NSTAGE - 1:
            # stage input for the next eval: z + alpha*k
            nc.vector.scalar_tensor_tensor(
                inTb[0:FD_P, js], cur[:, :], ev["alpha"], inTa[0:FD_P, js],
                op0=OP.mult, op1=OP.add,
            )
            # RK4 accumulator: acc = w*CUR + (state | acc)
            base = inTa[0:FD_P, js] if stage == 0 else acc[:, js]
            nc.vector.scalar_tensor_tensor(
                acc[:, js], cur[:, :], ev["w"], base,
                op0=OP.mult, op1=OP.add,
            )
        else:
            # step end: z_{n+1} = w3*CUR + acc, written into the state buffer
            nc.vector.scalar_tensor_tensor(
                inTa[0:FD_P, js], cur[:, :], ev["w"], acc[:, js],
                op0=OP.mult, op1=OP.add,
            )

    pend = deque()
    for evi in range(NEV):
        for uu in range(NJ):
            emit_front(evi, uu)
            pend.append((evi, uu))
            if len(pend) > LAG:
                emit_back(*pend.popleft())
    while pend:
        emit_back(*pend.popleft())

    # ---- finalize: out = -0.5*sum(z1^2) - 0.5*D*log(2pi) + delta_logp ----
    z1 = ept
    nc.vector.tensor_scalar(z1[:, :], inTa[0:DIM, :], b2c[:], None, op0=OP.add)
    zsq = ept
    nc.vector.tensor_tensor(zsq[:, :], z1[:, :], z1[:, :], op=OP.mult)
    outr = acc[0:1, :]  # dead fp32 row: keeps full fp32 output precision
    for j in range(NJ):
        js = slice(j * 512, (j + 1) * 512)
        pZ = fd_pool.tile([1, 512], f32, tag="fd")
        nc.tensor.matmul(pZ[:, :], onesZ[:], zsq[:, js], start=True, stop=True)
        nc.vector.scalar_tensor_tensor(
            outr[:, js], pZ[:, :], -0.5, inTa[DV : DV + 1, js],
            op0=OP.mult, op1=OP.add,
        )
    nc.gpsimd.dma_start(out_d, outr)


_COMPILED = {}


def _get_compiled():
    if "nc" in _COMPILED:
        return _COMPILED["nc"]
    from contextlib import ExitStack
    import concourse.tile as tile
    from concourse import bacc

    nc = bacc.Bacc("TRN2", target_bir_lowering=False, debug=False,
                   num_devices=NCORES)
    with tile.TileContext(nc) as tc, ExitStack() as ctx:
        build(nc, tc, ctx)
    nc.compile()
    _COMPILED["nc"] = nc
    return nc


def kernel(x, context, eps, W1, b1, W2, b2, steps):
    from concourse.bass_utils import run_bass_kernel_spmd

    assert int(steps) == 5, "kernel hardcodes the steps=5 schedule"
    in_maps = prep_host_inputs(x, context, eps, W1, b1, W2, b2)
    nc = _get_compiled()
    res = run_bass_kernel_spmd(nc, in_maps, list(range(NCORES)))
    out = np.concatenate(
        [res.results[i]["out"].reshape(NB, 1) for i in range(NCORES)], axis=0
    )
    return out.astype(np.float32)


if __name__ == "__main__":
    rng = np.random.default_rng(0)
    ins = dict(
        x=rng.standard_normal((B, DIM), dtype=np.float32),
        context=rng.standard_normal((B, COND), dtype=np.float32),
        eps=rng.standard_normal((B, DIM), dtype=np.float32),
        W1=(rng.standard_normal((KIN - 1, HID)) / np.sqrt(KIN - 1)).astype(np.float32),
        b1=np.zeros(HID, np.float32),
        W2=(rng.standard_normal((HID, DIM)) / np.sqrt(HID)).astype(np.float32),
        b2=np.zeros(DIM, np.float32),
        steps=5,
    )
    print(kernel(**ins)[:4])


# revision 12
# speedup vs baseline: 1.0715x; 1.0715x over previous
"""Trainium2 Bass kernel for CNF log-prob (nn_CNF_86019605004441).

Reference computation (per batch row b of B=32768):
  Integrate (z, logp) from t=1 to t=0 with 4 fixed RK4 steps (steps=5 ->
  4 intervals). Each RK4 stage evaluates
     f(t, z)   = tanh([z, ctx, t] @ W1 + b1) @ W2 + b2
     div(t, z) = eps^T J eps  (Hutchinson, exact via jvp)
  With h = tanh(a):  div = sum_j (1 - h_j^2) * t1_j * v_j
     where t1 = eps @ W1[:16]  and  v = eps @ W2^T  are eval-independent.
  Using u = t1*v and U = sum_j u_j:  div = U - S,  S = sum_j h_j^2 u_j.
  logp(x) = -0.5*sum(z1^2) - 0.5*16*log(2pi) + delta_logp.

Sharding: pure data parallel, batch 32768 -> 8 cores x 4096 rows.

On-core layout (features on partitions, batch on the free axis), v2:
  Two mm1 input buffers inTa/inTb [98, 4096] f32r: rows 0-15 z, 16-32
  scratch (row 32 = logp in inTa), 33-95 ctx, 96 ones, 97 last ctx row.
  inTa holds the step-start state (z_n, logp_n); stages 1-3 read/write
  inTb as the stage input (z + alpha*k).

  Per eval (16 total), per unit (8 units of 512 batch cols), pipelined
  with an emission lag so the in-order PE queue never head-blocks:
    mm1: 4 chunk matmuls [98,128]^T @ inT -> pa psum (2 half tiles)
    tanh -> h fp16 (ACT, 2 ops);  hh = h*h (DVE);  q = hh*u (DVE)
    mm2: f = W2^T h -> CUR[0:32], S = ones^T q -> CUR[32] (8 fp16
         matmuls into one 1-bank psum tile; no ACC matmuls)
    RK4 state updates as scalar_tensor_tensor on GPSIMD (Pool):
      stages 0-2: inTb = alpha*CUR + inTa ; acc = w*CUR + (inTa|acc)
      stage 3:    inTa = w3*CUR + acc     (z_{n+1}, logp_{n+1})
  (logp row: CUR[32] = +S; div = U - S telescopes the U term into the
   logp init, exactly as v1.)
Finalize: zsq = (z1 - b2)^2 ; colsum via ones-matmul ; out = -0.5*colsum
  + inTa[32].
"""

import sys
from collections import deque
import numpy as np

for _p in ("/opt/trn_rl_repo",):
    if _p not in sys.path:
        sys.path.insert(0, _p)

DIM, COND, HID = 16, 64, 512
B, NCORES = 32768, 8
NB = B // NCORES          # 4096 batch rows per core
P = 128                   # partitions
NCH = HID // P            # 4 hidden chunks
NJ = NB // 512            # 8 batch column groups (units)
NSCR = 17                 # scratch rows 16..32 (div lands at 32)
KIN = DIM + NSCR + COND + 1  # 98 stationary rows
FD_P = DIM + NSCR            # 33 = fd/state partition rows
CTX0 = DIM + NSCR            # ctx rows 33..95 + row 97 (96 is the ones row)
ONE_R = 96                   # ones row
DV = DIM + NSCR - 1          # 32 = divergence / logp row
NSTEPS, NSTAGE = 4, 4
NEV = NSTEPS * NSTAGE     # 16 rhs evaluations
LOG2PI = float(np.log(2.0 * np.pi))
LAG = 5                   # software-pipeline emission lag (units)
# hh engine per unit: pool 4/8, act 2.5/8, dve 1.5/8 (TensorScalarPtr is
# not a legal Pool opcode, so Pool pays the 0.42 Multiply efficiency)
HHE = ["pool", "act", "pool", "dve", "pool", "act", "pool", "x"]


def _schedule():
    """Per-eval (t, alpha_next, w, delta) for classic RK4, t:1->0, dt=-0.25."""
    ts = np.linspace(1.0, 0.0, NSTEPS + 1)
    evs = []
    for s in range(NSTEPS):
        t0 = float(ts[s])
        dt = float(ts[s + 1] - ts[s])
        dbase = s * dt
        evs.append(dict(t=t0, alpha=dt / 2, w=dt / 6, delta=dbase))
        evs.append(dict(t=t0 + dt / 2, alpha=dt / 2, w=dt / 3, delta=dbase + dt / 2))
        evs.append(dict(t=t0 + dt / 2, alpha=dt, w=dt / 3, delta=dbase + dt / 2))
        evs.append(dict(t=t0 + dt, alpha=None, w=dt / 6, delta=dbase + dt))
    return evs


def prep_host_inputs(x, context, eps, W1, b1, W2, b2):
    """Host-side layout prep (transposes + per-eval stationary weight packing).

    Returns the in_map dict for one core given that core's batch slice."""
    evs = _schedule()
    W1 = np.asarray(W1, np.float32)
    b1 = np.asarray(b1, np.float32)
    W2 = np.asarray(W2, np.float32)
    b2 = np.asarray(b2, np.float32)

    gz = W1[:DIM].T @ b2  # [512], the z-column correction for deferred b2
    W1v = np.zeros((KIN, NEV * NCH, P), np.float32)
    for i, ev in enumerate(evs):
        for c in range(NCH):
            sl = slice(c * P, (c + 1) * P)
            v = i * NCH + c
            W1v[0:DIM, v, :] = W1[0:DIM, sl]
            # rows DIM..DIM+NSCR-1 stay zero: scratch rows of inT
            W1v[CTX0:ONE_R, v, :] = W1[DIM : DIM + COND - 1, sl]
            W1v[KIN - 1, v, :] = W1[DIM + COND - 1, sl]
            W1v[ONE_R, v, :] = (
                ev["t"] * W1[DIM + COND, sl] + b1[sl] + ev["delta"] * gz[sl]
            )

    W2f16 = np.zeros((P, NCH, 32), np.float16)
    W2f16[:, :, :DIM] = W2.reshape(NCH, P, DIM).transpose(1, 0, 2).astype(np.float16)
    onesW = np.ones((P, 1), np.float16)
    W2T = np.ascontiguousarray(W2.T)  # [16, 512] for the v = eps@W2^T matmul
    b2c = (4 * (-0.25)) * b2.reshape(DIM, 1).astype(np.float32)  # D_final*b2

    def core_map(xs, cs, es):
        initT = np.zeros((KIN, NB), np.float32)
        initT[0:DIM] = xs.T
        initT[CTX0:ONE_R] = cs.T[0 : COND - 1]
        initT[KIN - 1] = cs.T[COND - 1]
        initT[ONE_R] = 1.0
        return {
            "initT": initT,                          # [98, NB]
            "W1z": np.ascontiguousarray(W1v[0:DIM, 0:NCH, :]),  # [16, 4, 128]
            "epsT": np.ascontiguousarray(es.T),     # [16, NB]
            "onesZ": np.ones((DIM, 1), np.float32),
            "W1v": W1v,                              # [98, 64, 128]
            "W2T": W2T,                              # [16, 512]
            "W2f16": W2f16,                          # [128, 4, 32]
            "onesW": onesW,                          # [128, 1]
            "b2c": b2c,                              # [16, 1]
        }

    return [
        core_map(
            np.asarray(x, np.float32)[i * NB : (i + 1) * NB],
            np.asarray(context, np.float32)[i * NB : (i + 1) * NB],
            np.asarray(eps, np.float32)[i * NB : (i + 1) * NB],
        )
        for i in range(NCORES)
    ]


def build(nc, tc, ctx):
    """Emit the kernel into TileContext tc (single SPMD program, all cores)."""
    import concourse.bass as bass
    from concourse import mybir

    f32 = mybir.dt.float32
    f32r = mybir.dt.float32r
    f16 = mybir.dt.float16
    AF = mybir.ActivationFunctionType
    OP = mybir.AluOpType
    evs = _schedule()

    initT = nc.dram_tensor("initT", [KIN, NB], f32r, kind="ExternalInput").ap()
    W1z_d = nc.dram_tensor("W1z", [DIM, NCH, P], f32r, kind="ExternalInput").ap()
    epsT = nc.dram_tensor("epsT", [DIM, NB], f32r, kind="ExternalInput").ap()
    onesZ_d = nc.dram_tensor("onesZ", [DIM, 1], f32r, kind="ExternalInput").ap()
    W1v_d = nc.dram_tensor("W1v", [KIN, NEV * NCH, P], f32r, kind="ExternalInput").ap()
    W2T_d = nc.dram_tensor("W2T", [DIM, HID], f32r, kind="ExternalInput").ap()
    W2f_d = nc.dram_tensor("W2f16", [P, NCH, 32], f16, kind="ExternalInput").ap()
    onesW_d = nc.dram_tensor("onesW", [P, 1], f16, kind="ExternalInput").ap()
    b2c_d = nc.dram_tensor("b2c", [DIM, 1], f32, kind="ExternalInput").ap()
    out_d = nc.dram_tensor("out", [1, NB], f32, kind="ExternalOutput").ap()

    const = ctx.enter_context(tc.tile_pool(name="const", bufs=1))
    state = ctx.enter_context(tc.tile_pool(name="state", bufs=1))
    work = ctx.enter_context(tc.tile_pool(name="work", bufs=4))
    pa_pool = ctx.enter_context(tc.tile_pool(name="pa", bufs=2, space="PSUM"))
    fd_pool = ctx.enter_context(tc.tile_pool(name="fd", bufs=4, space="PSUM"))

    # ---- persistent SBUF ----
    inTa = state.tile([KIN, NB], f32r)   # step-start state + ctx/ones
    inTb = state.tile([KIN, NB], f32r)   # stage input (z + alpha*k) + ctx/ones
    acc = state.tile([FD_P, NB], f32)    # RK4 weighted accumulator
    u = state.tile([P, NCH, NB], f16)
    W1v = const.tile([KIN, NEV * NCH, P], f32r)
    W2T = const.tile([DIM, HID], f32r)
    W2f = const.tile([P, NCH, 32], f16)
    onesW = const.tile([P, 1], f16)
    ones16 = const.tile([P, 1], f16)
    onesZ = const.tile([DIM, 1], f32r)
    b2c = const.tile([DIM, 1], f32)
    ept = const.tile([DIM, NB], f32r)
    W1z = const.tile([DIM, NCH, P], f32r)

    # small tensors first so the u-precompute starts immediately; the big
    # W1v/initT loads overlap with it
    nc.gpsimd.dma_start(ept[:], epsT)
    nc.gpsimd.dma_start(W1z[:], W1z_d)
    nc.gpsimd.dma_start(W2T[:], W2T_d)
    nc.gpsimd.dma_start(W2f[:], W2f_d)
    nc.gpsimd.dma_start(onesW[:], onesW_d)
    nc.gpsimd.dma_start(onesZ[:], onesZ_d)
    nc.gpsimd.dma_start(b2c[:], b2c_d)
    nc.gpsimd.dma_start(inTa[:, :], initT)
    nc.gpsimd.dma_start(W1v[:], W1v_d)
    nc.gpsimd.dma_start(inTb[:, :], initT)
    nc.vector.memset(ones16[:], 1.0)

    # ---- precompute u = (eps@W1z) * (eps@W2^T), transposed layout ----
    for qt in range(4):
        for c in range(NCH):
            js = slice(qt * (NB // 4), (qt + 1) * (NB // 4))
            pt1 = pa_pool.tile([P, 2, 512], f32, tag="pa")
            pt2 = pa_pool.tile([P, 2, 512], f32, tag="pa")
            for n in range(2):
                cs = slice((qt * 2 + n) * 512, (qt * 2 + n + 1) * 512)
                nc.tensor.matmul(
                    pt1[:, n, :], W1z[:, c, :], ept[:, cs], start=True, stop=True
                )
                nc.tensor.matmul(
                    pt2[:, n, :], W2T[:, c * P : (c + 1) * P], ept[:, cs],
                    start=True, stop=True,
                )
            usl = u[:, c, js].rearrange("p (a b) -> p a b", a=2)
            nc.scalar.activation(usl, pt1[:, :, :], AF.Copy)
            nc.vector.tensor_tensor(usl, usl, pt2[:, :, :], op=OP.mult)

    # ---- U = colsum(u) -> inTa row 32 = U - 0.5*DIM*log(2pi) ----
    for j in range(NJ):
        js = slice(j * 512, (j + 1) * 512)
        pU = fd_pool.tile([1, 512], f32, tag="fd")
        for c in range(NCH):
            nc.tensor.matmul(
                pU[:, :], ones16[:], u[:, c, js], start=(c == 0), stop=(c == NCH - 1)
            )
        nc.scalar.activation(
            inTa[DV : DV + 1, js], pU[:, :], AF.Copy, bias=-0.5 * DIM * LOG2PI
        )

    # ---- main loop: 16 evals x 8 units, software-pipelined emission ----
    hq = {}

    def emit_front(evi, uu):
        """mm1 -> tanh -> hh -> q for (eval evi, unit uu)."""
        js = slice(uu * 512, (uu + 1) * 512)
        src = inTa if evi % NSTAGE == 0 else inTb
        h = work.tile([P, NCH, 512], f16, tag="h", bufs=7)
        for half in range(2):
            pa = pa_pool.tile([P, 2, 512], f32, tag="pa")
            for n in range(2):
                c = half * 2 + n
                nc.tensor.matmul(
                    pa[:, n, :], W1v[:, evi * NCH + c, :], src[:, js],
                    start=True, stop=True,
                )
            nc.scalar.activation(
                h[:, half * 2 : half * 2 + 2, :], pa[:, :, :], AF.Tanh
            )
        # hh engine is balanced statically: Pool (SBUF-only op, so GPSIMD is
        # legal) takes half the units, ACT-Square ~2.5/8, DVE the rest; q
        # (tensor*tensor) runs on DVE in the 2x fp16 mode.
        hht = work.tile([P, NCH, 512], f16, tag="hh", bufs=4)
        he = HHE[uu] if HHE[uu] != "x" else ("act" if evi % 2 == 0 else "dve")
        if he == "act":
            nc.scalar.activation(hht[:, :, :], h[:, :, :], AF.Square)
        elif he == "pool":
            nc.gpsimd.tensor_tensor(hht[:, :, :], h[:, :, :], h[:, :, :], op=OP.mult)
        else:
            nc.vector.tensor_tensor(hht[:, :, :], h[:, :, :], h[:, :, :], op=OP.mult)
        q = work.tile([P, NCH, 512], f16, tag="q", bufs=7)
        nc.vector.tensor_tensor(q[:, :, :], hht[:, :, :], u[:, :, js], op=OP.mult)
        hq[(evi, uu)] = (h, q)

    def emit_back(evi, uu):
        """mm2 (f + S) -> RK4 state STTs for (eval evi, unit uu)."""
        js = slice(uu * 512, (uu + 1) * 512)
        h, q = hq.pop((evi, uu))
        cur = fd_pool.tile([FD_P, 512], f32, tag="fd")
        for c in range(NCH):
            st, sp = c == 0, c == NCH - 1
            nc.tensor.matmul(
                cur[0:32, :], W2f[:, c, :], h[:, c, :],
                start=st, stop=sp, skip_group_check=True,
            )
            nc.tensor.matmul(
                cur[DV : DV + 1, :], onesW[:, 0:1], q[:, c, :],
                start=st, stop=sp, skip_group_check=True,
            )
        ev = evs[evi]
        stage = evi % NSTAGE
        if stage < NSTAGE - 1:
            # stage input for the next eval: z + alpha*k
            nc.vector.scalar_tensor_tensor(
                inTb[0:FD_P, js], cur[:, :], ev["alpha"], inTa[0:FD_P, js],
                op0=OP.mult, op1=OP.add,
            )
            # RK4 accumulator: acc = w*CUR + (state | acc)
            base = inTa[0:FD_P, js] if stage == 0 else acc[:, js]
            nc.vector.scalar_tensor_tensor(
                acc[:, js], cur[:, :], ev["w"], base,
                op0=OP.mult, op1=OP.add,
            )
        else:
            # step end: z_{n+1} = w3*CUR + acc, written into the state buffer
            nc.vector.scalar_tensor_tensor(
                inTa[0:FD_P, js], cur[:, :], ev["w"], acc[:, js],
                op0=OP.mult, op1=OP.add,
            )

    pend = deque()
    for evi in range(NEV):
        for uu in range(NJ):
            if len(pend) >= LAG:
                emit_back(*pend.popleft())
            emit_front(evi, uu)
            pend.append((evi, uu))
    while pend:
        emit_back(*pend.popleft())

    # ---- finalize: out = -0.5*sum(z1^2) - 0.5*D*log(2pi) + delta_logp ----
    z1 = ept
    nc.vector.tensor_scalar(z1[:, :], inTa[0:DIM, :], b2c[:], None, op0=OP.add)
    zsq = ept
    nc.vector.tensor_tensor(zsq[:, :], z1[:, :], z1[:, :], op=OP.mult)
    outr = acc[0:1, :]  # dead fp32 row: keeps full fp32 output precision
    for j in range(NJ):
        js = slice(j * 512, (j + 1) * 512)
        pZ = fd_pool.tile([1, 512], f32, tag="fd")
        nc.tensor.matmul(pZ[:, :], onesZ[:], zsq[:, js], start=True, stop=True)
        nc.vector.scalar_tensor_tensor(
            outr[:, js], pZ[:, :], -0.5, inTa[DV : DV + 1, js],
            op0=OP.mult, op1=OP.add,
        )
    nc.gpsimd.dma_start(out_d, outr)


_COMPILED = {}


def _get_compiled():
    if "nc" in _COMPILED:
        return _COMPILED["nc"]
    from contextlib import ExitStack
    import concourse.tile as tile
    from concourse import bacc

    nc = bacc.Bacc("TRN2", target_bir_lowering=False, debug=False,
                   num_devices=NCORES)
    with tile.TileContext(nc) as tc, ExitStack() as ctx:
        build(nc, tc, ctx)
    nc.compile()
    _COMPILED["nc"] = nc
    return nc


def kernel(x, context, eps, W1, b1, W2, b2, steps):
    from concourse.bass_utils import run_bass_kernel_spmd

    assert int(steps) == 5, "kernel hardcodes the steps=5 schedule"
    in_maps = prep_host_inputs(x, context, eps, W1, b1, W2, b2)
    nc = _get_compiled()
    res = run_bass_kernel_spmd(nc, in_maps, list(range(NCORES)))
    out = np.concatenate(
        [res.results[i]["out"].reshape(NB, 1) for i in range(NCORES)], axis=0
    )
    return out.astype(np.float32)


if __name__ == "__main__":
    rng = np.random.default_rng(0)
    ins = dict(
        x=rng.standard_normal((B, DIM), dtype=np.float32),
        context=rng.standard_normal((B, COND), dtype=np.float32),
        eps=rng.standard_normal((B, DIM), dtype=np.float32),
        W1=(rng.standard_normal((KIN - 1, HID)) / np.sqrt(KIN - 1)).astype(np.float32),
        b1=np.zeros(HID, np.float32),
        W2=(rng.standard_normal((HID, DIM)) / np.sqrt(HID)).astype(np.float32),
        b2=np.zeros(DIM, np.float32),
        steps=5,
    )
    print(kernel(**ins)[:4])


# revision 15
# speedup vs baseline: 1.0854x; 1.0130x over previous
"""Trainium2 Bass kernel for CNF log-prob (nn_CNF_86019605004441).

Reference computation (per batch row b of B=32768):
  Integrate (z, logp) from t=1 to t=0 with 4 fixed RK4 steps (steps=5 ->
  4 intervals). Each RK4 stage evaluates
     f(t, z)   = tanh([z, ctx, t] @ W1 + b1) @ W2 + b2
     div(t, z) = eps^T J eps  (Hutchinson, exact via jvp)
  With h = tanh(a):  div = sum_j (1 - h_j^2) * t1_j * v_j
     where t1 = eps @ W1[:16]  and  v = eps @ W2^T  are eval-independent.
  Using u = t1*v and U = sum_j u_j:  div = U - S,  S = sum_j h_j^2 u_j.
  logp(x) = -0.5*sum(z1^2) - 0.5*16*log(2pi) + delta_logp.

Sharding: pure data parallel, batch 32768 -> 8 cores x 4096 rows.

On-core layout (features on partitions, batch on the free axis), v2:
  Two mm1 input buffers inTa/inTb [98, 4096] f32r: rows 0-15 z, 16-32
  scratch (row 32 = logp in inTa), 33-95 ctx, 96 ones, 97 last ctx row.
  inTa holds the step-start state (z_n, logp_n); stages 1-3 read/write
  inTb as the stage input (z + alpha*k).

  Per eval (16 total), per unit (8 units of 512 batch cols), pipelined
  with an emission lag so the in-order PE queue never head-blocks:
    mm1: 4 chunk matmuls [98,128]^T @ inT -> pa psum (2 half tiles)
    tanh -> h fp16 (ACT, 2 ops);  hh = h*h (DVE);  q = hh*u (DVE)
    mm2: f = W2^T h -> CUR[0:32], S = ones^T q -> CUR[32] (8 fp16
         matmuls into one 1-bank psum tile; no ACC matmuls)
    RK4 state updates as scalar_tensor_tensor on GPSIMD (Pool):
      stages 0-2: inTb = alpha*CUR + inTa ; acc = w*CUR + (inTa|acc)
      stage 3:    inTa = w3*CUR + acc     (z_{n+1}, logp_{n+1})
  (logp row: CUR[32] = +S; div = U - S telescopes the U term into the
   logp init, exactly as v1.)
Finalize: zsq = (z1 - b2)^2 ; colsum via ones-matmul ; out = -0.5*colsum
  + inTa[32].
"""

import sys
from collections import deque
import numpy as np

for _p in ("/opt/trn_rl_repo",):
    if _p not in sys.path:
        sys.path.insert(0, _p)

DIM, COND, HID = 16, 64, 512
B, NCORES = 32768, 8
NB = B // NCORES          # 4096 batch rows per core
P = 128                   # partitions
NCH = HID // P            # 4 hidden chunks
NJ = NB // 512            # 8 batch column groups (units)
NSCR = 17                 # scratch rows 16..32 (div lands at 32)
KIN = DIM + NSCR + COND + 1  # 98 stationary rows
FD_P = DIM + NSCR            # 33 = fd/state partition rows
CTX0 = DIM + NSCR            # ctx rows 33..95 + row 97 (96 is the ones row)
ONE_R = 96                   # ones row
DV = DIM + NSCR - 1          # 32 = divergence / logp row
NSTEPS, NSTAGE = 4, 4
NEV = NSTEPS * NSTAGE     # 16 rhs evaluations
LOG2PI = float(np.log(2.0 * np.pi))
TRACE_LABELS = {}  # instruction name -> semantic label (for trace analysis)


def _lbl(inst, label):
    try:
        TRACE_LABELS[inst.ins.name] = label
    except Exception:
        pass
    return inst
LAG_F = 2                 # emission lag for f matmuls (need h)
LAG_D = 5                 # emission lag for div matmuls + state STTs (need q)
# hh engine per unit: pool 4/8, act 2.5/8, dve 1.5/8 (TensorScalarPtr is
# not a legal Pool opcode, so Pool pays the 0.42 Multiply efficiency)
HHE = ["pool", "act", "pool", "dve", "pool", "act", "pool", "x"]


def _schedule():
    """Per-eval (t, alpha_next, w, delta) for classic RK4, t:1->0, dt=-0.25."""
    ts = np.linspace(1.0, 0.0, NSTEPS + 1)
    evs = []
    for s in range(NSTEPS):
        t0 = float(ts[s])
        dt = float(ts[s + 1] - ts[s])
        dbase = s * dt
        evs.append(dict(t=t0, alpha=dt / 2, w=dt / 6, delta=dbase))
        evs.append(dict(t=t0 + dt / 2, alpha=dt / 2, w=dt / 3, delta=dbase + dt / 2))
        evs.append(dict(t=t0 + dt / 2, alpha=dt, w=dt / 3, delta=dbase + dt / 2))
        evs.append(dict(t=t0 + dt, alpha=None, w=dt / 6, delta=dbase + dt))
    return evs


def prep_host_inputs(x, context, eps, W1, b1, W2, b2):
    """Host-side layout prep (transposes + per-eval stationary weight packing).

    Returns the in_map dict for one core given that core's batch slice."""
    evs = _schedule()
    W1 = np.asarray(W1, np.float32)
    b1 = np.asarray(b1, np.float32)
    W2 = np.asarray(W2, np.float32)
    b2 = np.asarray(b2, np.float32)

    gz = W1[:DIM].T @ b2  # [512], the z-column correction for deferred b2
    W1v = np.zeros((KIN, NEV * NCH, P), np.float32)
    for i, ev in enumerate(evs):
        for c in range(NCH):
            sl = slice(c * P, (c + 1) * P)
            v = i * NCH + c
            W1v[0:DIM, v, :] = W1[0:DIM, sl]
            # rows DIM..DIM+NSCR-1 stay zero: scratch rows of inT
            W1v[CTX0:ONE_R, v, :] = W1[DIM : DIM + COND - 1, sl]
            W1v[KIN - 1, v, :] = W1[DIM + COND - 1, sl]
            W1v[ONE_R, v, :] = (
                ev["t"] * W1[DIM + COND, sl] + b1[sl] + ev["delta"] * gz[sl]
            )

    W2f16 = np.zeros((P, NCH, 32), np.float16)
    W2f16[:, :, :DIM] = W2.reshape(NCH, P, DIM).transpose(1, 0, 2).astype(np.float16)
    onesW = np.ones((P, 1), np.float16)
    W2T = np.ascontiguousarray(W2.T)  # [16, 512] for the v = eps@W2^T matmul
    b2c = (4 * (-0.25)) * b2.reshape(DIM, 1).astype(np.float32)  # D_final*b2

    def core_map(xs, cs, es):
        initT = np.zeros((KIN, NB), np.float32)
        initT[0:DIM] = xs.T
        initT[CTX0:ONE_R] = cs.T[0 : COND - 1]
        initT[KIN - 1] = cs.T[COND - 1]
        initT[ONE_R] = 1.0
        return {
            "initT": initT,                          # [98, NB]
            "W1z": np.ascontiguousarray(W1v[0:DIM, 0:NCH, :]),  # [16, 4, 128]
            "epsT": np.ascontiguousarray(es.T),     # [16, NB]
            "onesZ": np.ones((DIM, 1), np.float32),
            "W1v": W1v,                              # [98, 64, 128]
            "W2T": W2T,                              # [16, 512]
            "W2f16": W2f16,                          # [128, 4, 32]
            "onesW": onesW,                          # [128, 1]
            "b2c": b2c,                              # [16, 1]
        }

    return [
        core_map(
            np.asarray(x, np.float32)[i * NB : (i + 1) * NB],
            np.asarray(context, np.float32)[i * NB : (i + 1) * NB],
            np.asarray(eps, np.float32)[i * NB : (i + 1) * NB],
        )
        for i in range(NCORES)
    ]


def build(nc, tc, ctx):
    """Emit the kernel into TileContext tc (single SPMD program, all cores)."""
    import concourse.bass as bass
    from concourse import mybir

    f32 = mybir.dt.float32
    f32r = mybir.dt.float32r
    f16 = mybir.dt.float16
    AF = mybir.ActivationFunctionType
    OP = mybir.AluOpType
    evs = _schedule()

    initT = nc.dram_tensor("initT", [KIN, NB], f32r, kind="ExternalInput").ap()
    W1z_d = nc.dram_tensor("W1z", [DIM, NCH, P], f32r, kind="ExternalInput").ap()
    epsT = nc.dram_tensor("epsT", [DIM, NB], f32r, kind="ExternalInput").ap()
    onesZ_d = nc.dram_tensor("onesZ", [DIM, 1], f32r, kind="ExternalInput").ap()
    W1v_d = nc.dram_tensor("W1v", [KIN, NEV * NCH, P], f32r, kind="ExternalInput").ap()
    W2T_d = nc.dram_tensor("W2T", [DIM, HID], f32r, kind="ExternalInput").ap()
    W2f_d = nc.dram_tensor("W2f16", [P, NCH, 32], f16, kind="ExternalInput").ap()
    onesW_d = nc.dram_tensor("onesW", [P, 1], f16, kind="ExternalInput").ap()
    b2c_d = nc.dram_tensor("b2c", [DIM, 1], f32, kind="ExternalInput").ap()
    out_d = nc.dram_tensor("out", [1, NB], f32, kind="ExternalOutput").ap()

    const = ctx.enter_context(tc.tile_pool(name="const", bufs=1))
    state = ctx.enter_context(tc.tile_pool(name="state", bufs=1))
    work = ctx.enter_context(tc.tile_pool(name="work", bufs=4))
    pa_pool = ctx.enter_context(tc.tile_pool(name="pa", bufs=2, space="PSUM"))
    fd_pool = ctx.enter_context(tc.tile_pool(name="fd", bufs=4, space="PSUM"))

    # ---- persistent SBUF ----
    inTa = state.tile([KIN, NB], f32r)   # step-start state + ctx/ones
    inTb = state.tile([KIN, NB], f32r)   # stage input (z + alpha*k) + ctx/ones
    acc = state.tile([FD_P, NB], f32)    # RK4 weighted accumulator
    u = state.tile([P, NCH, NB], f16)
    W1v = const.tile([KIN, NEV * NCH, P], f32r)
    W2T = const.tile([DIM, HID], f32r)
    W2f = const.tile([P, NCH, 32], f16)
    onesW = const.tile([P, 1], f16)
    ones16 = const.tile([P, 1], f16)
    onesZ = const.tile([DIM, 1], f32r)
    b2c = const.tile([DIM, 1], f32)
    ept = const.tile([DIM, NB], f32r)
    W1z = const.tile([DIM, NCH, P], f32r)

    # small tensors first so the u-precompute starts immediately; the big
    # W1v/initT loads overlap with it
    nc.gpsimd.dma_start(ept[:], epsT)
    nc.gpsimd.dma_start(W1z[:], W1z_d)
    nc.gpsimd.dma_start(W2T[:], W2T_d)
    nc.gpsimd.dma_start(W2f[:], W2f_d)
    nc.gpsimd.dma_start(onesW[:], onesW_d)
    nc.gpsimd.dma_start(onesZ[:], onesZ_d)
    nc.gpsimd.dma_start(b2c[:], b2c_d)
    nc.gpsimd.dma_start(inTa[:, :], initT)
    nc.gpsimd.dma_start(W1v[:], W1v_d)
    nc.gpsimd.dma_start(inTb[:, :], initT)
    nc.vector.memset(ones16[:], 1.0)

    # ---- precompute u = (eps@W1z) * (eps@W2^T), transposed layout ----
    for qt in range(4):
        for c in range(NCH):
            js = slice(qt * (NB // 4), (qt + 1) * (NB // 4))
            pt1 = pa_pool.tile([P, 2, 512], f32, tag="pa")
            pt2 = pa_pool.tile([P, 2, 512], f32, tag="pa")
            for n in range(2):
                cs = slice((qt * 2 + n) * 512, (qt * 2 + n + 1) * 512)
                nc.tensor.matmul(
                    pt1[:, n, :], W1z[:, c, :], ept[:, cs], start=True, stop=True
                )
                nc.tensor.matmul(
                    pt2[:, n, :], W2T[:, c * P : (c + 1) * P], ept[:, cs],
                    start=True, stop=True,
                )
            usl = u[:, c, js].rearrange("p (a b) -> p a b", a=2)
            nc.scalar.activation(usl, pt1[:, :, :], AF.Copy)
            nc.vector.tensor_tensor(usl, usl, pt2[:, :, :], op=OP.mult)

    # ---- U = colsum(u) -> inTa row 32 = U - 0.5*DIM*log(2pi) ----
    for j in range(NJ):
        js = slice(j * 512, (j + 1) * 512)
        pU = fd_pool.tile([1, 512], f32, tag="fd")
        for c in range(NCH):
            nc.tensor.matmul(
                pU[:, :], ones16[:], u[:, c, js], start=(c == 0), stop=(c == NCH - 1)
            )
        nc.scalar.activation(
            inTa[DV : DV + 1, js], pU[:, :], AF.Copy, bias=-0.5 * DIM * LOG2PI
        )

    # ---- main loop: 16 evals x 8 units, software-pipelined emission ----
    hq = {}

    def emit_front(evi, uu):
        """mm1 -> tanh -> hh -> q for (eval evi, unit uu)."""
        js = slice(uu * 512, (uu + 1) * 512)
        src = inTa if evi % NSTAGE == 0 else inTb
        h = work.tile([P, NCH, 512], f16, tag="h", bufs=5)
        for half in range(2):
            pa = pa_pool.tile([P, 2, 512], f32, tag="pa")
            for n in range(2):
                c = half * 2 + n
                _lbl(nc.tensor.matmul(
                    pa[:, n, :], W1v[:, evi * NCH + c, :], src[:, js],
                    start=True, stop=True,
                ), f"mm1.u{uu}.c{half*2+n}")
            _lbl(nc.scalar.activation(
                h[:, half * 2 : half * 2 + 2, :], pa[:, :, :], AF.Tanh
            ), f"tanh.u{uu}.h{half}")
        # hh engine is balanced statically: Pool (SBUF-only op, so GPSIMD is
        # legal) takes half the units, ACT-Square ~2.5/8, DVE the rest; q
        # (tensor*tensor) runs on DVE in the 2x fp16 mode.
        hht = work.tile([P, NCH, 512], f16, tag="hh", bufs=4)
        he = HHE[uu] if HHE[uu] != "x" else ("act" if evi % 2 == 0 else "dve")
        if he == "act":
            _lbl(nc.scalar.activation(hht[:, :, :], h[:, :, :], AF.Square),
                 f"hhA.u{uu}")
        elif he == "pool":
            _lbl(nc.gpsimd.tensor_tensor(
                hht[:, :, :], h[:, :, :], h[:, :, :], op=OP.mult), f"hhP.u{uu}")
        else:
            _lbl(nc.vector.tensor_tensor(
                hht[:, :, :], h[:, :, :], h[:, :, :], op=OP.mult), f"hhD.u{uu}")
        q = work.tile([P, NCH, 512], f16, tag="q", bufs=7)
        _lbl(nc.vector.tensor_tensor(
            q[:, :, :], hht[:, :, :], u[:, :, js], op=OP.mult), f"q.u{uu}")
        hq[(evi, uu)] = (h, q)

    curs = {}

    def emit_f(evi, uu):
        """f = W2^T h matmuls (only need h -> small lag)."""
        h, q = hq[(evi, uu)]
        cur = fd_pool.tile([FD_P, 512], f32, tag="fd")
        curs[(evi, uu)] = cur
        for c in range(NCH):
            st, sp = c == 0, c == NCH - 1
            _lbl(nc.tensor.matmul(
                cur[0:32, :], W2f[:, c, :], h[:, c, :],
                start=st, stop=sp, skip_group_check=True,
            ), f"f.u{uu}.c{c}")

    def emit_div_state(evi, uu):
        """S = ones^T q matmuls + RK4 state STTs (need q -> larger lag)."""
        js = slice(uu * 512, (uu + 1) * 512)
        h, q = hq.pop((evi, uu))
        cur = curs.pop((evi, uu))
        for c in range(NCH):
            st, sp = c == 0, c == NCH - 1
            _lbl(nc.tensor.matmul(
                cur[DV : DV + 1, :], onesW[:, 0:1], q[:, c, :],
                start=st, stop=sp, skip_group_check=True,
            ), f"div.u{uu}.c{c}")
        ev = evs[evi]
        stage = evi % NSTAGE
        if stage < NSTAGE - 1:
            # stage input for the next eval: z + alpha*k
            _lbl(nc.vector.scalar_tensor_tensor(
                inTb[0:FD_P, js], cur[:, :], ev["alpha"], inTa[0:FD_P, js],
                op0=OP.mult, op1=OP.add,
            ), f"ztmp.u{uu}")
            # RK4 accumulator: acc = w*CUR + (state | acc)
            base = inTa[0:FD_P, js] if stage == 0 else acc[:, js]
            _lbl(nc.vector.scalar_tensor_tensor(
                acc[:, js], cur[:, :], ev["w"], base,
                op0=OP.mult, op1=OP.add,
            ), f"accS.u{uu}")
        else:
            # step end: z_{n+1} = w3*CUR + acc, written into the state buffer
            _lbl(nc.vector.scalar_tensor_tensor(
                inTa[0:FD_P, js], cur[:, :], ev["w"], acc[:, js],
                op0=OP.mult, op1=OP.add,
            ), f"fin.u{uu}")

    pf, pd = deque(), deque()
    for evi in range(NEV):
        for uu in range(NJ):
            if len(pd) >= LAG_D - LAG_F:
                emit_div_state(*pd.popleft())
            if len(pf) >= LAG_F:
                it = pf.popleft()
                emit_f(*it)
                pd.append(it)
            emit_front(evi, uu)
            pf.append((evi, uu))
    while pf:
        if pd:
            emit_div_state(*pd.popleft())
        it = pf.popleft()
        emit_f(*it)
        pd.append(it)
    while pd:
        emit_div_state(*pd.popleft())

    # ---- finalize: out = -0.5*sum(z1^2) - 0.5*D*log(2pi) + delta_logp ----
    z1 = ept
    nc.vector.tensor_scalar(z1[:, :], inTa[0:DIM, :], b2c[:], None, op0=OP.add)
    zsq = ept
    nc.vector.tensor_tensor(zsq[:, :], z1[:, :], z1[:, :], op=OP.mult)
    outr = acc[0:1, :]  # dead fp32 row: keeps full fp32 output precision
    for j in range(NJ):
        js = slice(j * 512, (j + 1) * 512)
        pZ = fd_pool.tile([1, 512], f32, tag="fd")
        nc.tensor.matmul(pZ[:, :], onesZ[:], zsq[:, js], start=True, stop=True)
        nc.vector.scalar_tensor_tensor(
            outr[:, js], pZ[:, :], -0.5, inTa[DV : DV + 1, js],
            op0=OP.mult, op1=OP.add,
        )
    nc.gpsimd.dma_start(out_d, outr)


_COMPILED = {}


def _get_compiled():
    if "nc" in _COMPILED:
        return _COMPILED["nc"]
    from contextlib import ExitStack
    import concourse.tile as tile
    from concourse import bacc

    nc = bacc.Bacc("TRN2", target_bir_lowering=False, debug=False,
                   num_devices=NCORES)
    with tile.TileContext(nc) as tc, ExitStack() as ctx:
        build(nc, tc, ctx)
    nc.compile()
    _COMPILED["nc"] = nc
    return nc


def kernel(x, context, eps, W1, b1, W2, b2, steps):
    from concourse.bass_utils import run_bass_kernel_spmd

    assert int(steps) == 5, "kernel hardcodes the steps=5 schedule"
    in_maps = prep_host_inputs(x, context, eps, W1, b1, W2, b2)
    nc = _get_compiled()
    res = run_bass_kernel_spmd(nc, in_maps, list(range(NCORES)))
    out = np.concatenate(
        [res.results[i]["out"].reshape(NB, 1) for i in range(NCORES)], axis=0
    )
    return out.astype(np.float32)


if __name__ == "__main__":
    rng = np.random.default_rng(0)
    ins = dict(
        x=rng.standard_normal((B, DIM), dtype=np.float32),
        context=rng.standard_normal((B, COND), dtype=np.float32),
        eps=rng.standard_normal((B, DIM), dtype=np.float32),
        W1=(rng.standard_normal((KIN - 1, HID)) / np.sqrt(KIN - 1)).astype(np.float32),
        b1=np.zeros(HID, np.float32),
        W2=(rng.standard_normal((HID, DIM)) / np.sqrt(HID)).astype(np.float32),
        b2=np.zeros(DIM, np.float32),
        steps=5,
    )
    print(kernel(**ins)[:4])
